# revision 28
# baseline (speedup 1.0000x reference)
"""Multi-head attention (B=4, L=2048, D=768, H=12) on 8 Trainium2 NeuronCores.

Sharding: (batch, head-group). Core c handles batch c//2 and heads
6*(c%2) .. 6*(c%2)+6.  Each core computes its 6 heads' attention output and
the partial output projection y_part = AO @ Wo[rows]; the host sums the two
partials per batch, rescales, and adds biases.  No collectives.

Per-core pipeline (fp16 operands; fp32 PSUM accumulate):
  A. QKV projections in fp8e4m3 DoubleRow with an exact 3-term residual
     decomposition (all terms at x256 scale, one PSUM group each):
        256*x@W = x8@W256 + xr16@W16 + x8@Wr
     where x8=fp8(x), xr16=fp8(16(x-x8)), W256=fp8(256W), W16=fp8(16W),
     Wr=fp8(256W - W256).  Eviction converts to fp16 (Q adds 256*bq; bk is
     dropped -- softmax shift-invariant; bv/bo folded on the host).
  B. Scores in fp16: S^T[k,q] = K_h @ Q_h^T -> PSUM [128, 512];
     p = exp(s/8 - 2.5) in fp16, computed on ACT (real exp) or DVE
     (Schraudolph: the uint16 bits ARE the fp16 pattern; uint16 saturation
     at 0 handles underflow), balanced by running engine-load accumulators.
     PV runs query-major: pv[q, qc8, 0:64] accumulated over the 16 key
     chunks of one query-half; denominators via ones-matmuls in their own
     PSUM bank.  AO = 256*attn in fp16.
  C. Two query-half sweeps (qc 0-7 then 8-15) over all heads, so the AO
     transpose (XBAR) + output projection + y DMA of the first half overlap
     the second sweep.  y is fp16 at 256x; host: (y0+y1)/256 + bv@Wo + bo.
  D. A long run of tiny dummy matmuls at kernel start keeps PE busy through
     the p-state ramp until the first input DMAs land, so all real matmuls
     run at full clock (PE idle gaps reset the ramp).
"""

import numpy as np
import ml_dtypes

import concourse.mybir as mybir
import concourse.tile as tile
from concourse import bacc
from concourse.bass_utils import run_bass_kernel_spmd

F32 = mybir.dt.float32
F16 = mybir.dt.float16
U16 = mybir.dt.uint16
FP8 = mybir.dt.float8e4
P = 128
B, L, D, H = 4, 2048, 768, 12
HD = 64                    # head dim
HL = H // 2                # heads per core = 6
HO = HL * HD               # local feature dim = 384
KC = D // P                # contraction chunks over D = 6
CP = KC // 2               # DoubleRow contraction chunk-pairs = 3
LC = L // P                # key chunks = 16
MC = HO // P               # feature chunks = 3
QC = LC                    # query chunks = 16
DR = mybir.MatmulPerfMode.DoubleRow

LOG2E = 1.4426950408889634
SHIFT = 2.5
# scores psum = (256Q)*(256K) = 65536 * s;  p = exp(s/8 - SHIFT)
ACT_SCALE = 0.125 / 65536.0
SCH_A = 1024.0 * LOG2E / (8.0 * 65536.0)
SCH_C = -60.0
SCH_B = 15 * 1024 - SHIFT * LOG2E * 1024.0 + SCH_C

PV_LAG = 7                 # PV trails scores by this many lk-units
N_WARMUP = 60             # tiny dummy matmuls bridging the p-state ramp

# engine time models (ns) for load balancing
ACT_EXP = 570.0
DVE_EXP = 658.0

_NC = None


def s512(i):
    return slice(i * 512, (i + 1) * 512)


def build():
    nc = bacc.Bacc("TRN2", target_bir_lowering=False, debug=False)

    # x blocks of 512 queries, contiguous per (partition, block)
    x8 = nc.dram_tensor("x8", [P, 4, CP, 2, 512], FP8, kind="ExternalInput")
    xr = nc.dram_tensor("xr", [P, 4, CP, 2, 512], FP8, kind="ExternalInput")
    w_names = []
    for t in ("q", "k", "v"):
        for v in ("a", "b", "r"):       # a=W256, b=W16, r=Wr
            w_names.append(f"w{t}{v}")
    w_dram = {
        n: nc.dram_tensor(n, [P, CP * 2 * HO], FP8, kind="ExternalInput")
        for n in w_names
    }
    wo = nc.dram_tensor("wo", [P, MC * D], F16, kind="ExternalInput")
    bq = nc.dram_tensor("bq", [HO], F32, kind="ExternalInput")
    y = nc.dram_tensor("y", [L, D], F16, kind="ExternalOutput")

    with tile.TileContext(nc) as tc:
        with tc.tile_pool(name="static", bufs=1) as static:
            qT_tiles = [static.tile([P, L], F16, name=f"qT{m}") for m in range(MC)]
            kT_tiles = [static.tile([P, L], F16, name=f"kT{m}") for m in range(MC)]
            v_sb = static.tile([P, LC, HL, HD], F16)
            ones_sb = static.tile([P, 1], F16)
            ao_q = static.tile([P, QC, HL, HD], F16)      # query-major AO
            ao_t = static.tile([P, QC, MC, P], F16)       # feature-major AO
            bq_sb = static.tile([P, MC], F32)
            shift_sb = static.tile([P, 1], F32)
            dummy_sb = static.tile([P, 64], F16)
            dummy_ps_out = static.tile([P, 1], F32)

            nc.vector.memset(dummy_sb[:], 0.0)

            with (
                tc.tile_pool(name="xpool", bufs=1) as xpool,
                tc.tile_pool(name="wpool", bufs=1) as wpool,
            ):
                x8_sb = xpool.tile([P, 4, CP, 2, 512], FP8, name="x8")
                xr_sb = xpool.tile([P, 4, CP, 2, 512], FP8, name="xr")
                w_sb = {
                    n: wpool.tile([P, CP, 2, HO], FP8, name=n) for n in w_names
                }
                wo_sb = wpool.tile([P, MC, D], F16, name="wo")

                def w_dma(n):
                    nc.sync.dma_start(
                        w_sb[n][:],
                        w_dram[n].ap().rearrange(
                            "p (c t h) -> p c t h", c=CP, t=2
                        ),
                    )

                # DMA order matches prelude consumption:
                #   Q00 K00 Q01 K01 V0 V1 Q02 K02 Q03 K03
                w_dma("wqa")
                for cp in range(CP):
                    nc.sync.dma_start(x8_sb[:, 0, cp], x8[:, 0, cp])
                w_dma("wqr")
                nc.sync.dma_start(xr_sb[:, 0], xr[:, 0])
                w_dma("wqb")
                for v in ("a", "b", "r"):
                    w_dma(f"wk{v}")
                nc.sync.dma_start(x8_sb[:, 1], x8[:, 1])
                nc.sync.dma_start(xr_sb[:, 1], xr[:, 1])
                for v in ("a", "b", "r"):
                    w_dma(f"wv{v}")
                for j in (2, 3):
                    nc.sync.dma_start(x8_sb[:, j], x8[:, j])
                    nc.sync.dma_start(xr_sb[:, j], xr[:, j])
                nc.sync.dma_start(bq_sb[:], bq.ap().rearrange("(c p) -> p c", p=P))
                nc.sync.dma_start(
                    wo_sb[:, :, :], wo.ap().rearrange("p (c d) -> p c d", c=MC)
                )

                with (
                    tc.tile_pool(name="ppool", bufs=12) as ppool,
                    tc.tile_pool(name="rpool", bufs=4) as rpool,
                    tc.tile_pool(name="ypool", bufs=4) as ypool,
                    tc.tile_pool(name="sps", bufs=6, space="PSUM") as sps,
                    tc.tile_pool(name="pvps", bufs=1, space="PSUM") as pvps,
                    tc.tile_pool(name="dnps", bufs=1, space="PSUM") as dnps,
                ):
                    # ---- PE p-state warm-up: tiny dummy matmuls keep PE
                    # busy until the first input DMAs land (idle resets
                    # the ramp, so bridge the whole window) ----
                    dn_full = dnps.tile([P, 2, 256], F32, name="dn")

                    def dummies(n):
                        # keep PE busy through DMA waits: accumulate zeros
                        # into a spare never-started region of the dn bank
                        for _ in range(n):
                            nc.tensor.matmul(
                                dn_full[0:64, 0, 128:192],
                                dummy_sb[:, :],
                                dummy_sb[:, :],
                                start=False,
                                stop=False,
                                skip_group_check=True,
                            )

                    dummies(N_WARMUP)
                    # deferred setup (not needed until the first exp/PV)
                    nc.vector.memset(dn_full[:, :, 0:8], 0.0)
                    nc.vector.memset(ones_sb[:], 0.0625)  # dn = sum(p)/16
                    nc.vector.memset(shift_sb[:], -SHIFT)
                    # preload the exp activation table while DMAs run
                    nc.scalar.activation(
                        out=dummy_ps_out[:],
                        in_=shift_sb[:],
                        func=mybir.ActivationFunctionType.Exp,
                    )
                    # deferred setup (not needed until the first exp/PV)
                    nc.vector.memset(dn_full[:, :, 0:8], 0.0)
                    nc.vector.memset(ones_sb[:], 0.0625)  # dn = sum(p)/16
                    nc.vector.memset(shift_sb[:], -SHIFT)
                    # preload the exp activation table while DMAs run
                    nc.scalar.activation(
                        out=dummy_ps_out[:],
                        in_=shift_sb[:],
                        func=mybir.ActivationFunctionType.Exp,
                    )

                    # engine-balance accumulators (ns)
                    eng_load = {"A": 0.0, "D": 0.0}

                    def pick_engine(act_cost, dve_cost):
                        if eng_load["A"] + act_cost <= eng_load["D"] + dve_cost:
                            eng_load["A"] += act_cost
                            return "A"
                        eng_load["D"] += dve_cost
                        return "D"

                    def emit_proj(w3, msl, j, moving_x, out_w, out_cb,
                                  nout=512):
                        """3-term fp8 DR projection into one PSUM group."""
                        ps = sps.tile([P, 512], F32, tag="s", name="pj")
                        wa, wb, wr = w3
                        first = True
                        for wt, xt in ((wa, x8_sb), (wr, x8_sb), (wb, xr_sb)):
                            for cp in range(CP):
                                if moving_x:
                                    lhs = wt[:, cp, :, msl]
                                    rhs = xt[:, j, cp, :, :]
                                else:
                                    lhs = xt[:, j, cp, :, msl]
                                    rhs = wt[:, cp, :, :]
                                nc.tensor.matmul(
                                    ps[:, 0:nout],
                                    lhs,
                                    rhs,
                                    start=first,
                                    stop=(wt is wb and cp == CP - 1),
                                    perf_mode=DR,
                                    skip_group_check=True,
                                )
                                first = False
                        out_cb(ps)

                    def do_qk(which, m, j):
                        w3 = tuple(w_sb[f"w{which}{v}"] for v in ("a", "b", "r"))

                        def evict(ps):
                            out_ap = (qT_tiles if which == "q" else kT_tiles)[m][
                                :, s512(j)
                            ]
                            e = pick_engine(570.0, 658.0)
                            if which == "q":
                                if e == "A":
                                    nc.scalar.activation(
                                        out=out_ap,
                                        in_=ps[:, :],
                                        func=mybir.ActivationFunctionType.Identity,
                                        bias=bq_sb[:, m : m + 1],
                                    )
                                else:
                                    nc.vector.tensor_scalar(
                                        out_ap,
                                        ps[:, :],
                                        1.0,
                                        bq_sb[:, m : m + 1],
                                        mybir.AluOpType.mult,
                                        mybir.AluOpType.add,
                                    )
                            else:
                                if e == "A":
                                    nc.scalar.activation(
                                        out=out_ap,
                                        in_=ps[:, :],
                                        func=mybir.ActivationFunctionType.Copy,
                                    )
                                else:
                                    nc.vector.tensor_copy(out_ap, ps[:, :])

                        emit_proj(w3, slice(m * P, (m + 1) * P), j, True, None,
                                  evict)

                    def do_v(l):
                        w3 = tuple(w_sb[f"wv{v}"] for v in ("a", "b", "r"))

                        def evict(ps):
                            e = pick_engine(463.0, 525.0)
                            dst = v_sb[:, l, :, :]
                            src = ps[:, 0:HO].rearrange("p (h d) -> p h d", d=HD)
                            if e == "A":
                                nc.scalar.activation(
                                    out=dst, in_=src,
                                    func=mybir.ActivationFunctionType.Copy,
                                )
                            else:
                                nc.vector.tensor_copy(dst, src)

                        emit_proj(w3, slice((l % 4) * P, (l % 4 + 1) * P),
                                  l // 4, False, None, evict, nout=HO)

                    def emit_scores_exp(hl, lk, q4):
                        pc, odd = hl // 2, hl % 2
                        r0 = odd * HD
                        s_t = sps.tile([P, 512], F32, tag="s", name="sc")
                        nc.tensor.matmul(
                            s_t[:, :],
                            kT_tiles[pc][r0 : r0 + HD, lk * P : (lk + 1) * P],
                            qT_tiles[pc][r0 : r0 + HD, s512(q4)],
                            start=True,
                            stop=True,
                        )
                        p_t = ppool.tile([P, 512], F16, tag="p", name="pt")
                        if pick_engine(ACT_EXP, DVE_EXP) == "A":
                            nc.scalar.activation(
                                out=p_t[:, :],
                                in_=s_t[:, :],
                                func=mybir.ActivationFunctionType.Exp,
                                bias=shift_sb[:, 0:1],
                                scale=ACT_SCALE,
                            )
                        else:
                            nc.vector.tensor_scalar(
                                p_t[:, :].bitcast(U16),
                                s_t[:, :],
                                SCH_A,
                                SCH_B,
                                mybir.AluOpType.mult,
                                mybir.AluOpType.add,
                            )
                        return p_t

                    def emit_pv(pv, hl, lk, q4, p_t, sweep):
                        first = lk == 0 and q4 % 2 == 0
                        last = lk == LC - 1
                        for jj in range(4):
                            qc8 = (q4 % 2) * 4 + jj
                            nc.tensor.matmul(
                                pv[:, qc8, :],
                                p_t[:, jj * P : (jj + 1) * P],
                                v_sb[:, lk, hl, :],
                                start=(first and jj == 0),
                                stop=last,
                                skip_group_check=True,
                            )
                            nc.tensor.matmul(
                                dn_full[:, hl % 2, qc8 : qc8 + 1],
                                p_t[:, jj * P : (jj + 1) * P],
                                ones_sb[:, :],
                                start=False,
                                stop=last,
                                skip_group_check=True,
                            )

                    def evict_ao(pv, hl, sweep, split=False):
                        """recip + batched scaled eviction (256*attn, fp16)."""
                        rstage = rpool.tile([P, 8], F32, tag="r")
                        nc.vector.reciprocal(rstage[:, :], dn_full[:, hl % 2, 0:8])
                        eng_load["D"] += 135.0
                        if hl + 2 < HL or sweep == 0:
                            nc.vector.memset(dn_full[:, hl % 2, 0:8], 0.0)
                        qc0 = sweep * 8
                        groups = ((0, 4), (4, 8)) if split else ((0, 8),)
                        for g0, g1 in groups:
                            rb = rstage[:, g0:g1, None].broadcast_to(
                                (P, g1 - g0, HD))
                            nc.vector.scalar_tensor_tensor(
                                ao_q[:, qc0 + g0 : qc0 + g1, hl, :],
                                pv[:, g0:g1, :],
                                0.0625,
                                rb,
                                mybir.AluOpType.mult,
                                mybir.AluOpType.mult,
                            )
                            eng_load["D"] += 658.0 / len(groups)
                            if split:
                                quad_transpose(2 + (g0 // 4))

                    def quad_transpose(g):
                        nc.sync.dma_start_transpose(
                            ao_t[:, 4 * g : 4 * g + 4, :, :],
                            ao_q[:, 4 * g : 4 * g + 4, :, :],
                        )

                    def do_outproj(m):
                        y_t = ypool.tile([P, D], F16, tag="yt")
                        for n0, nsz in ((0, 512), (512, 256)):
                            ps = sps.tile([P, 512], F32, tag="s", name="yp")
                            for c in range(MC):
                                nc.tensor.matmul(
                                    ps[:, 0:nsz],
                                    ao_t[:, m, c, :],
                                    wo_sb[:, c, n0 : n0 + nsz],
                                    start=(c == 0),
                                    stop=(c == MC - 1),
                                )
                            e = pick_engine(
                                (nsz + 172) * 0.8333, (nsz + 120) * 1.0417)
                            if e == "A":
                                nc.scalar.activation(
                                    out=y_t[:, n0 : n0 + nsz], in_=ps[:, 0:nsz],
                                    func=mybir.ActivationFunctionType.Copy,
                                )
                            else:
                                nc.vector.tensor_copy(
                                    y_t[:, n0 : n0 + nsz], ps[:, 0:nsz])
                        yeng = nc.sync if m % 2 == 0 else nc.scalar
                        yeng.dma_start(y[m * P : (m + 1) * P, :], y_t[:])

                    # ---------------- prelude ----------------
                    do_qk("q", 0, 0)
                    do_qk("k", 0, 0)
                    for job in (
                        lambda: do_qk("q", 0, 1), lambda: do_qk("k", 0, 1),
                        lambda: do_v(0), lambda: do_v(1),
                        lambda: do_qk("q", 0, 2), lambda: do_qk("k", 0, 2),
                        lambda: do_qk("q", 0, 3), lambda: do_qk("k", 0, 3),
                    ):
                        dummies(22)
                        job()

                    prejobs = {}
                    for j in range(4):                     # qT/kT chunk 1
                        prejobs.setdefault(4 * j, []).append(
                            lambda j=j: do_qk("q", 1, j))
                        prejobs.setdefault(4 * j + 2, []).append(
                            lambda j=j: do_qk("k", 1, j))
                    for l in range(2, LC):                 # V chunks 2-15
                        prejobs.setdefault(l - 1, []).append(lambda l=l: do_v(l))
                    for j in range(4):                     # qT/kT chunk 2
                        prejobs.setdefault(32 + 4 * j, []).append(
                            lambda j=j: do_qk("q", 2, j))
                        prejobs.setdefault(32 + 4 * j + 2, []).append(
                            lambda j=j: do_qk("k", 2, j))
                    # sweep B: transposes of qc 0-7 (after the (s1,h0) pump
                    # evicts (s0,h5)), then outproj jobs
                    t0 = 192 + PV_LAG + 3
                    prejobs.setdefault(t0, []).append(lambda: quad_transpose(0))
                    prejobs.setdefault(t0 + 2, []).append(
                        lambda: quad_transpose(1))
                    for i in range(8):                     # outproj qc 0-7
                        prejobs.setdefault(t0 + 12 + 16 * i, []).append(
                            lambda i=i: do_outproj(i))

                    units = []
                    for sweep in range(2):
                        for hl in range(HL):
                            if sweep == 1 and hl == HL - 1:
                                # final head: q4h-major so qc 8-11 finish
                                # early and their evict/transpose/outproj
                                # overlap the qc 12-15 stream
                                for q4h in range(2):
                                    for lk in range(LC):
                                        units.append((sweep, hl, lk, q4h))
                            else:
                                for lk in range(LC):
                                    for q4h in range(2):
                                        units.append((sweep, hl, lk, q4h))
                    pend = []
                    pv_state = {"tile": None, "key": None}

                    def evict_ao_part(pv, hl, sweep, g0, g1):
                        rstage = rpool.tile([P, 4], F32, tag="r")
                        nc.vector.reciprocal(
                            rstage[:, :], dn_full[:, hl % 2, g0:g1])
                        qc0 = sweep * 8
                        rb = rstage[:, :, None].broadcast_to((P, g1 - g0, HD))
                        nc.vector.scalar_tensor_tensor(
                            ao_q[:, qc0 + g0 : qc0 + g1, hl, :],
                            pv[:, g0:g1, :],
                            0.0625,
                            rb,
                            mybir.AluOpType.mult,
                            mybir.AluOpType.mult,
                        )

                    def pump_pv(entry):
                        hl, lk, q4, p_t, sweep = entry
                        key = (sweep, hl)
                        if pv_state["key"] != key:
                            if pv_state["tile"] is not None:
                                osweep, ohl = pv_state["key"]
                                evict_ao(pv_state["tile"], ohl, osweep)
                            pv_state["tile"] = pvps.tile(
                                [P, 8, HD], F32, tag="pv", name="pv")
                            pv_state["key"] = key
                        emit_pv(pv_state["tile"], hl, lk, q4, p_t, sweep)
                        if (sweep, hl, lk, q4) == (1, HL - 1, LC - 1, 2):
                            # qc 8-11 of the final head are complete: evict
                            # and transpose now, overlapping the qc 12-15
                            # stream
                            evict_ao_part(pv_state["tile"], HL - 1, 1, 0, 4)
                            quad_transpose(2)

                    for u, (sweep, hl, lk, q4h) in enumerate(units):
                        q4 = 2 * sweep + q4h
                        if u < 16 and u % 2 == 1:
                            dummies(8)
                        for job in prejobs.get(u, ()):
                            job()
                        p_t = emit_scores_exp(hl, lk, q4)
                        pend.append((hl, lk, q4, p_t, sweep))
                        while len(pend) > PV_LAG:
                            pump_pv(pend.pop(0))
                    while pend:
                        pump_pv(pend.pop(0))
                    # tail: evict qc 12-15, then outproj 8-11 (transpose(2)
                    # already done) runs during transpose(3)'s DMA latency
                    evict_ao_part(pv_state["tile"], HL - 1, 1, 4, 8)
                    quad_transpose(3)
                    for i in range(8, QC):
                        do_outproj(i)

    nc.compile()
    return nc


def _get_nc():
    global _NC
    if _NC is None:
        _NC = build()
    return _NC


E4NP = ml_dtypes.float8_e4m3


def _dr_rows_x(a):
    """[768, 2048] -> [128, 4, 3, 2, 512]: query-block-major DR layout;
    row (cp, t, p) holds input row cp*256 + t*128 + p."""
    return np.ascontiguousarray(
        a.reshape(CP, 2, P, 4, 512).transpose(2, 3, 0, 1, 4)
    )


def _w_tensors(W):
    """W [768, 384] fp32 -> (W256, W16, Wr) fp8, flattened [128, 3*2*384]
    with row (cp, t, p) holding input row cp*256 + t*128 + p."""
    Wa = (256.0 * W).astype(E4NP)
    Wb = (16.0 * W).astype(E4NP)
    Wr = (256.0 * W - Wa.astype(np.float32)).astype(E4NP)
    return tuple(
        np.ascontiguousarray(
            t.reshape(CP, 2, P, HO).transpose(2, 0, 1, 3).reshape(P, CP * 2 * HO)
        )
        for t in (Wa, Wb, Wr)
    )


def kernel(**inputs) -> np.ndarray:
    x = np.asarray(inputs["x"], dtype=np.float32)
    Wq = np.asarray(inputs["Wq"], dtype=np.float32)
    Wk = np.asarray(inputs["Wk"], dtype=np.float32)
    Wv = np.asarray(inputs["Wv"], dtype=np.float32)
    Wo = np.asarray(inputs["Wo"], dtype=np.float32)
    bq = np.asarray(inputs["bq"], dtype=np.float32)
    bv = np.asarray(inputs["bv"], dtype=np.float32)
    bo = np.asarray(inputs["bo"], dtype=np.float32)

    nc = _get_nc()

    in_maps = []
    for c in range(8):
        b, hg = c // 2, c % 2
        cs = slice(hg * HO, (hg + 1) * HO)
        xT = np.ascontiguousarray(x[b].T)               # [768, 2048]
        x8f = xT.astype(E4NP)
        xrf = (16.0 * (xT - x8f.astype(np.float32))).astype(E4NP)
        m = {"x8": _dr_rows_x(x8f), "xr": _dr_rows_x(xrf)}
        for t, W in (("q", Wq), ("k", Wk), ("v", Wv)):
            Ws = W[:, cs]
            for v, arr in zip(("a", "b", "r"), _w_tensors(Ws)):
                m[f"w{t}{v}"] = arr
        Wos = Wo[cs, :]                                  # [384, 768]
        m["wo"] = np.ascontiguousarray(
            Wos.reshape(MC, P, D).transpose(1, 0, 2).reshape(P, MC * D)
        ).astype(np.float16)
        m["bq"] = np.ascontiguousarray(256.0 * bq[cs])
        in_maps.append(m)

    res = run_bass_kernel_spmd(nc, in_maps, core_ids=list(range(8)))
    bias_full = bv @ Wo + bo
    out = np.empty((B, L, D), dtype=np.float32)
    for b in range(B):
        out[b] = (
            res.results[2 * b]["y"].astype(np.float32)
            + res.results[2 * b + 1]["y"].astype(np.float32)
        ) / 256.0 + bias_full
    return out


# revision 29
# speedup vs baseline: 1.0013x; 1.0013x over previous
"""Multi-head attention (B=4, L=2048, D=768, H=12) on 8 Trainium2 NeuronCores.

Sharding: (batch, head-group). Core c handles batch c//2 and heads
6*(c%2) .. 6*(c%2)+6.  Each core computes its 6 heads' attention output and
the partial output projection y_part = AO @ Wo[rows]; the host sums the two
partials per batch, rescales, and adds biases.  No collectives.

Per-core pipeline (fp16 operands; fp32 PSUM accumulate):
  A. QKV projections in fp8e4m3 DoubleRow with an exact 3-term residual
     decomposition (all terms at x256 scale, one PSUM group each):
        256*x@W = x8@W256 + xr16@W16 + x8@Wr
     where x8=fp8(x), xr16=fp8(16(x-x8)), W256=fp8(256W), W16=fp8(16W),
     Wr=fp8(256W - W256).  Eviction converts to fp16 (Q adds 256*bq; bk is
     dropped -- softmax shift-invariant; bv/bo folded on the host).
  B. Scores in fp16: S^T[k,q] = K_h @ Q_h^T -> PSUM [128, 512];
     p = exp(s/8 - 2.5) in fp16, computed on ACT (real exp) or DVE
     (Schraudolph: the uint16 bits ARE the fp16 pattern; uint16 saturation
     at 0 handles underflow), balanced by running engine-load accumulators.
     PV runs query-major: pv[q, qc8, 0:64] accumulated over the 16 key
     chunks of one query-half; denominators via ones-matmuls in their own
     PSUM bank.  AO = 256*attn in fp16.
  C. Two query-half sweeps (qc 0-7 then 8-15) over all heads, so the AO
     transpose (XBAR) + output projection + y DMA of the first half overlap
     the second sweep.  y is fp16 at 256x; host: (y0+y1)/256 + bv@Wo + bo.
  D. A long run of tiny dummy matmuls at kernel start keeps PE busy through
     the p-state ramp until the first input DMAs land, so all real matmuls
     run at full clock (PE idle gaps reset the ramp).
"""

import numpy as np
import ml_dtypes

import concourse.mybir as mybir
import concourse.tile as tile
from concourse import bacc
from concourse.bass_utils import run_bass_kernel_spmd

F32 = mybir.dt.float32
F16 = mybir.dt.float16
U16 = mybir.dt.uint16
FP8 = mybir.dt.float8e4
P = 128
B, L, D, H = 4, 2048, 768, 12
HD = 64                    # head dim
HL = H // 2                # heads per core = 6
HO = HL * HD               # local feature dim = 384
KC = D // P                # contraction chunks over D = 6
CP = KC // 2               # DoubleRow contraction chunk-pairs = 3
LC = L // P                # key chunks = 16
MC = HO // P               # feature chunks = 3
QC = LC                    # query chunks = 16
DR = mybir.MatmulPerfMode.DoubleRow

LOG2E = 1.4426950408889634
SHIFT = 2.5
# scores psum = (256Q)*(256K) = 65536 * s;  p = exp(s/8 - SHIFT)
ACT_SCALE = 0.125 / 65536.0
SCH_A = 1024.0 * LOG2E / (8.0 * 65536.0)
SCH_C = -60.0
SCH_B = 15 * 1024 - SHIFT * LOG2E * 1024.0 + SCH_C

PV_LAG = 7                 # PV trails scores by this many lk-units
N_WARMUP = 160             # tiny dummy matmuls bridging the p-state ramp

# engine time models (ns) for load balancing
ACT_EXP = 570.0
DVE_EXP = 658.0

_NC = None


def s512(i):
    return slice(i * 512, (i + 1) * 512)


def build():
    nc = bacc.Bacc("TRN2", target_bir_lowering=False, debug=False)

    # x blocks of 512 queries, contiguous per (partition, block)
    x8 = nc.dram_tensor("x8", [P, 4, CP, 2, 512], FP8, kind="ExternalInput")
    xr = nc.dram_tensor("xr", [P, 4, CP, 2, 512], FP8, kind="ExternalInput")
    w_names = []
    for t in ("q", "k", "v"):
        for v in ("a", "b", "r"):       # a=W256, b=W16, r=Wr
            w_names.append(f"w{t}{v}")
    w_dram = {
        n: nc.dram_tensor(n, [P, CP * 2 * HO], FP8, kind="ExternalInput")
        for n in w_names
    }
    wo = nc.dram_tensor("wo", [P, MC * D], F16, kind="ExternalInput")
    bq = nc.dram_tensor("bq", [HO], F32, kind="ExternalInput")
    y = nc.dram_tensor("y", [L, D], F16, kind="ExternalOutput")

    with tile.TileContext(nc) as tc:
        with tc.tile_pool(name="static", bufs=1) as static:
            qT_tiles = [static.tile([P, L], F16, name=f"qT{m}") for m in range(MC)]
            kT_tiles = [static.tile([P, L], F16, name=f"kT{m}") for m in range(MC)]
            v_sb = static.tile([P, LC, HL, HD], F16)
            ones_sb = static.tile([P, 1], F16)
            ao_q = static.tile([P, QC, HL, HD], F16)      # query-major AO
            ao_t = static.tile([P, QC, MC, P], F16)       # feature-major AO
            bq_sb = static.tile([P, MC], F32)
            shift_sb = static.tile([P, 1], F32)
            dummy_sb = static.tile([P, 64], F16)
            dummy_ps_out = static.tile([P, 1], F32)

            nc.vector.memset(dummy_sb[:], 0.0)

            with (
                tc.tile_pool(name="xpool", bufs=1) as xpool,
                tc.tile_pool(name="wpool", bufs=1) as wpool,
            ):
                x8_sb = xpool.tile([P, 4, CP, 2, 512], FP8, name="x8")
                xr_sb = xpool.tile([P, 4, CP, 2, 512], FP8, name="xr")
                w_sb = {
                    n: wpool.tile([P, CP, 2, HO], FP8, name=n) for n in w_names
                }
                wo_sb = wpool.tile([P, MC, D], F16, name="wo")

                def w_dma(n):
                    nc.sync.dma_start(
                        w_sb[n][:],
                        w_dram[n].ap().rearrange(
                            "p (c t h) -> p c t h", c=CP, t=2
                        ),
                    )

                # DMA order matches prelude consumption:
                #   Q00 K00 Q01 K01 V0 V1 Q02 K02 Q03 K03
                w_dma("wqa")
                nc.sync.dma_start(x8_sb[:, 0], x8[:, 0])
                nc.sync.dma_start(xr_sb[:, 0], xr[:, 0])
                w_dma("wqb")
                w_dma("wqr")
                for v in ("a", "b", "r"):
                    w_dma(f"wk{v}")
                nc.sync.dma_start(x8_sb[:, 1], x8[:, 1])
                nc.sync.dma_start(xr_sb[:, 1], xr[:, 1])
                for v in ("a", "b", "r"):
                    w_dma(f"wv{v}")
                for j in (2, 3):
                    nc.sync.dma_start(x8_sb[:, j], x8[:, j])
                    nc.sync.dma_start(xr_sb[:, j], xr[:, j])
                nc.sync.dma_start(bq_sb[:], bq.ap().rearrange("(c p) -> p c", p=P))
                nc.sync.dma_start(
                    wo_sb[:, :, :], wo.ap().rearrange("p (c d) -> p c d", c=MC)
                )

                with (
                    tc.tile_pool(name="ppool", bufs=12) as ppool,
                    tc.tile_pool(name="rpool", bufs=4) as rpool,
                    tc.tile_pool(name="ypool", bufs=4) as ypool,
                    tc.tile_pool(name="sps", bufs=6, space="PSUM") as sps,
                    tc.tile_pool(name="pvps", bufs=1, space="PSUM") as pvps,
                    tc.tile_pool(name="dnps", bufs=1, space="PSUM") as dnps,
                ):
                    # ---- PE p-state warm-up: tiny dummy matmuls keep PE
                    # busy until the first input DMAs land (idle resets
                    # the ramp, so bridge the whole window) ----
                    dn_full = dnps.tile([P, 2, 256], F32, name="dn")
                    nc.vector.memset(dn_full[0:64, 0, 128:192], 0.0)

                    def dummies(n):
                        # keep PE busy through DMA waits: accumulate zeros
                        # into a spare never-started region of the dn bank
                        for _ in range(n):
                            nc.tensor.matmul(
                                dn_full[0:64, 0, 128:192],
                                dummy_sb[:, :],
                                dummy_sb[:, :],
                                start=False,
                                stop=False,
                                skip_group_check=True,
                            )

                    dummies(N_WARMUP)
                    # deferred setup (not needed until the first exp/PV)
                    nc.vector.memset(dn_full[:, :, 0:8], 0.0)
                    nc.vector.memset(ones_sb[:], 0.0625)  # dn = sum(p)/16
                    nc.vector.memset(shift_sb[:], -SHIFT)
                    # preload the exp activation table while DMAs run
                    nc.scalar.activation(
                        out=dummy_ps_out[:],
                        in_=shift_sb[:],
                        func=mybir.ActivationFunctionType.Exp,
                    )
                    # deferred setup (not needed until the first exp/PV)
                    nc.vector.memset(dn_full[:, :, 0:8], 0.0)
                    nc.vector.memset(ones_sb[:], 0.0625)  # dn = sum(p)/16
                    nc.vector.memset(shift_sb[:], -SHIFT)
                    # preload the exp activation table while DMAs run
                    nc.scalar.activation(
                        out=dummy_ps_out[:],
                        in_=shift_sb[:],
                        func=mybir.ActivationFunctionType.Exp,
                    )

                    # engine-balance accumulators (ns)
                    eng_load = {"A": 0.0, "D": 0.0}

                    def pick_engine(act_cost, dve_cost):
                        if eng_load["A"] + act_cost <= eng_load["D"] + dve_cost:
                            eng_load["A"] += act_cost
                            return "A"
                        eng_load["D"] += dve_cost
                        return "D"

                    def emit_proj(w3, msl, j, moving_x, out_w, out_cb,
                                  nout=512):
                        """3-term fp8 DR projection into one PSUM group."""
                        ps = sps.tile([P, 512], F32, tag="s", name="pj")
                        wa, wb, wr = w3
                        first = True
                        for wt, xt in ((wa, x8_sb), (wb, xr_sb), (wr, x8_sb)):
                            for cp in range(CP):
                                if moving_x:
                                    lhs = wt[:, cp, :, msl]
                                    rhs = xt[:, j, cp, :, :]
                                else:
                                    lhs = xt[:, j, cp, :, msl]
                                    rhs = wt[:, cp, :, :]
                                nc.tensor.matmul(
                                    ps[:, 0:nout],
                                    lhs,
                                    rhs,
                                    start=first,
                                    stop=(wt is wr and cp == CP - 1),
                                    perf_mode=DR,
                                    skip_group_check=True,
                                )
                                first = False
                        out_cb(ps)

                    def do_qk(which, m, j):
                        w3 = tuple(w_sb[f"w{which}{v}"] for v in ("a", "b", "r"))

                        def evict(ps):
                            out_ap = (qT_tiles if which == "q" else kT_tiles)[m][
                                :, s512(j)
                            ]
                            e = pick_engine(570.0, 658.0)
                            if which == "q":
                                if e == "A":
                                    nc.scalar.activation(
                                        out=out_ap,
                                        in_=ps[:, :],
                                        func=mybir.ActivationFunctionType.Identity,
                                        bias=bq_sb[:, m : m + 1],
                                    )
                                else:
                                    nc.vector.tensor_scalar(
                                        out_ap,
                                        ps[:, :],
                                        1.0,
                                        bq_sb[:, m : m + 1],
                                        mybir.AluOpType.mult,
                                        mybir.AluOpType.add,
                                    )
                            else:
                                if e == "A":
                                    nc.scalar.activation(
                                        out=out_ap,
                                        in_=ps[:, :],
                                        func=mybir.ActivationFunctionType.Copy,
                                    )
                                else:
                                    nc.vector.tensor_copy(out_ap, ps[:, :])

                        emit_proj(w3, slice(m * P, (m + 1) * P), j, True, None,
                                  evict)

                    def do_v(l):
                        w3 = tuple(w_sb[f"wv{v}"] for v in ("a", "b", "r"))

                        def evict(ps):
                            e = pick_engine(463.0, 525.0)
                            dst = v_sb[:, l, :, :]
                            src = ps[:, 0:HO].rearrange("p (h d) -> p h d", d=HD)
                            if e == "A":
                                nc.scalar.activation(
                                    out=dst, in_=src,
                                    func=mybir.ActivationFunctionType.Copy,
                                )
                            else:
                                nc.vector.tensor_copy(dst, src)

                        emit_proj(w3, slice((l % 4) * P, (l % 4 + 1) * P),
                                  l // 4, False, None, evict, nout=HO)

                    def emit_scores_exp(hl, lk, q4):
                        pc, odd = hl // 2, hl % 2
                        r0 = odd * HD
                        s_t = sps.tile([P, 512], F32, tag="s", name="sc")
                        nc.tensor.matmul(
                            s_t[:, :],
                            kT_tiles[pc][r0 : r0 + HD, lk * P : (lk + 1) * P],
                            qT_tiles[pc][r0 : r0 + HD, s512(q4)],
                            start=True,
                            stop=True,
                        )
                        p_t = ppool.tile([P, 512], F16, tag="p", name="pt")
                        if pick_engine(ACT_EXP, DVE_EXP) == "A":
                            nc.scalar.activation(
                                out=p_t[:, :],
                                in_=s_t[:, :],
                                func=mybir.ActivationFunctionType.Exp,
                                bias=shift_sb[:, 0:1],
                                scale=ACT_SCALE,
                            )
                        else:
                            nc.vector.tensor_scalar(
                                p_t[:, :].bitcast(U16),
                                s_t[:, :],
                                SCH_A,
                                SCH_B,
                                mybir.AluOpType.mult,
                                mybir.AluOpType.add,
                            )
                        return p_t

                    def emit_pv(pv, hl, lk, q4, p_t, sweep):
                        first = lk == 0 and q4 % 2 == 0
                        last = lk == LC - 1
                        for jj in range(4):
                            qc8 = (q4 % 2) * 4 + jj
                            nc.tensor.matmul(
                                pv[:, qc8, :],
                                p_t[:, jj * P : (jj + 1) * P],
                                v_sb[:, lk, hl, :],
                                start=(first and jj == 0),
                                stop=last,
                                skip_group_check=True,
                            )
                            nc.tensor.matmul(
                                dn_full[:, hl % 2, qc8 : qc8 + 1],
                                p_t[:, jj * P : (jj + 1) * P],
                                ones_sb[:, :],
                                start=False,
                                stop=last,
                                skip_group_check=True,
                            )

                    def evict_ao(pv, hl, sweep, split=False):
                        """recip + batched scaled eviction (256*attn, fp16)."""
                        rstage = rpool.tile([P, 8], F32, tag="r")
                        nc.vector.reciprocal(rstage[:, :], dn_full[:, hl % 2, 0:8])
                        eng_load["D"] += 135.0
                        if hl + 2 < HL or sweep == 0:
                            nc.vector.memset(dn_full[:, hl % 2, 0:8], 0.0)
                        qc0 = sweep * 8
                        groups = ((0, 4), (4, 8)) if split else ((0, 8),)
                        for g0, g1 in groups:
                            rb = rstage[:, g0:g1, None].broadcast_to(
                                (P, g1 - g0, HD))
                            nc.vector.scalar_tensor_tensor(
                                ao_q[:, qc0 + g0 : qc0 + g1, hl, :],
                                pv[:, g0:g1, :],
                                0.0625,
                                rb,
                                mybir.AluOpType.mult,
                                mybir.AluOpType.mult,
                            )
                            eng_load["D"] += 658.0 / len(groups)
                            if split:
                                quad_transpose(2 + (g0 // 4))

                    def quad_transpose(g):
                        nc.sync.dma_start_transpose(
                            ao_t[:, 4 * g : 4 * g + 4, :, :],
                            ao_q[:, 4 * g : 4 * g + 4, :, :],
                        )

                    def do_outproj(m):
                        y_t = ypool.tile([P, D], F16, tag="yt")
                        for n0, nsz in ((0, 512), (512, 256)):
                            ps = sps.tile([P, 512], F32, tag="s", name="yp")
                            for c in range(MC):
                                nc.tensor.matmul(
                                    ps[:, 0:nsz],
                                    ao_t[:, m, c, :],
                                    wo_sb[:, c, n0 : n0 + nsz],
                                    start=(c == 0),
                                    stop=(c == MC - 1),
                                )
                            e = pick_engine(
                                (nsz + 172) * 0.8333, (nsz + 120) * 1.0417)
                            if e == "A":
                                nc.scalar.activation(
                                    out=y_t[:, n0 : n0 + nsz], in_=ps[:, 0:nsz],
                                    func=mybir.ActivationFunctionType.Copy,
                                )
                            else:
                                nc.vector.tensor_copy(
                                    y_t[:, n0 : n0 + nsz], ps[:, 0:nsz])
                        yeng = nc.sync if m % 2 == 0 else nc.scalar
                        yeng.dma_start(y[m * P : (m + 1) * P, :], y_t[:])

                    # ---------------- prelude ----------------
                    do_qk("q", 0, 0)
                    do_qk("k", 0, 0)
                    for job in (
                        lambda: do_qk("q", 0, 1), lambda: do_qk("k", 0, 1),
                        lambda: do_v(0), lambda: do_v(1),
                        lambda: do_qk("q", 0, 2), lambda: do_qk("k", 0, 2),
                        lambda: do_qk("q", 0, 3), lambda: do_qk("k", 0, 3),
                    ):
                        dummies(10)
                        job()

                    prejobs = {}
                    for j in range(4):                     # qT/kT chunk 1
                        prejobs.setdefault(4 * j, []).append(
                            lambda j=j: do_qk("q", 1, j))
                        prejobs.setdefault(4 * j + 2, []).append(
                            lambda j=j: do_qk("k", 1, j))
                    for l in range(2, LC):                 # V chunks 2-15
                        prejobs.setdefault(l - 1, []).append(lambda l=l: do_v(l))
                    for j in range(4):                     # qT/kT chunk 2
                        prejobs.setdefault(32 + 4 * j, []).append(
                            lambda j=j: do_qk("q", 2, j))
                        prejobs.setdefault(32 + 4 * j + 2, []).append(
                            lambda j=j: do_qk("k", 2, j))
                    # sweep B: transposes of qc 0-7 (after the (s1,h0) pump
                    # evicts (s0,h5)), then outproj jobs
                    t0 = 192 + PV_LAG + 3
                    prejobs.setdefault(t0, []).append(lambda: quad_transpose(0))
                    prejobs.setdefault(t0 + 2, []).append(
                        lambda: quad_transpose(1))
                    for i in range(8):                     # outproj qc 0-7
                        prejobs.setdefault(t0 + 12 + 16 * i, []).append(
                            lambda i=i: do_outproj(i))

                    units = []
                    for sweep in range(2):
                        for hl in range(HL):
                            if sweep == 1 and hl == HL - 1:
                                # final head: q4h-major so qc 8-11 finish
                                # early and their evict/transpose/outproj
                                # overlap the qc 12-15 stream
                                for q4h in range(2):
                                    for lk in range(LC):
                                        units.append((sweep, hl, lk, q4h))
                            else:
                                for lk in range(LC):
                                    for q4h in range(2):
                                        units.append((sweep, hl, lk, q4h))
                    pend = []
                    pv_state = {"tile": None, "key": None}

                    def evict_ao_part(pv, hl, sweep, g0, g1):
                        rstage = rpool.tile([P, 4], F32, tag="r")
                        nc.vector.reciprocal(
                            rstage[:, :], dn_full[:, hl % 2, g0:g1])
                        qc0 = sweep * 8
                        rb = rstage[:, :, None].broadcast_to((P, g1 - g0, HD))
                        nc.vector.scalar_tensor_tensor(
                            ao_q[:, qc0 + g0 : qc0 + g1, hl, :],
                            pv[:, g0:g1, :],
                            0.0625,
                            rb,
                            mybir.AluOpType.mult,
                            mybir.AluOpType.mult,
                        )

                    def pump_pv(entry):
                        hl, lk, q4, p_t, sweep = entry
                        key = (sweep, hl)
                        if pv_state["key"] != key:
                            if pv_state["tile"] is not None:
                                osweep, ohl = pv_state["key"]
                                evict_ao(pv_state["tile"], ohl, osweep)
                            pv_state["tile"] = pvps.tile(
                                [P, 8, HD], F32, tag="pv", name="pv")
                            pv_state["key"] = key
                        emit_pv(pv_state["tile"], hl, lk, q4, p_t, sweep)
                        if (sweep, hl, lk, q4) == (1, HL - 1, LC - 1, 2):
                            # qc 8-11 of the final head are complete: evict
                            # and transpose now, overlapping the qc 12-15
                            # stream
                            evict_ao_part(pv_state["tile"], HL - 1, 1, 0, 4)
                            quad_transpose(2)

                    for u, (sweep, hl, lk, q4h) in enumerate(units):
                        q4 = 2 * sweep + q4h
                        if u < 16 and u % 2 == 1:
                            dummies(8)
                        for job in prejobs.get(u, ()):
                            job()
                        p_t = emit_scores_exp(hl, lk, q4)
                        pend.append((hl, lk, q4, p_t, sweep))
                        while len(pend) > PV_LAG:
                            pump_pv(pend.pop(0))
                    while pend:
                        pump_pv(pend.pop(0))
                    # tail: evict qc 12-15, then outproj 8-11 (transpose(2)
                    # already done) runs during transpose(3)'s DMA latency
                    evict_ao_part(pv_state["tile"], HL - 1, 1, 4, 8)
                    quad_transpose(3)
                    for i in range(8, QC):
                        do_outproj(i)

    nc.compile()
    return nc


def _get_nc():
    global _NC
    if _NC is None:
        _NC = build()
    return _NC


E4NP = ml_dtypes.float8_e4m3


def _dr_rows_x(a):
    """[768, 2048] -> [128, 4, 3, 2, 512]: query-block-major DR layout;
    row (cp, t, p) holds input row cp*256 + t*128 + p."""
    return np.ascontiguousarray(
        a.reshape(CP, 2, P, 4, 512).transpose(2, 3, 0, 1, 4)
    )


def _w_tensors(W):
    """W [768, 384] fp32 -> (W256, W16, Wr) fp8, flattened [128, 3*2*384]
    with row (cp, t, p) holding input row cp*256 + t*128 + p."""
    Wa = (256.0 * W).astype(E4NP)
    Wb = (16.0 * W).astype(E4NP)
    Wr = (256.0 * W - Wa.astype(np.float32)).astype(E4NP)
    return tuple(
        np.ascontiguousarray(
            t.reshape(CP, 2, P, HO).transpose(2, 0, 1, 3).reshape(P, CP * 2 * HO)
        )
        for t in (Wa, Wb, Wr)
    )


def kernel(**inputs) -> np.ndarray:
    x = np.asarray(inputs["x"], dtype=np.float32)
    Wq = np.asarray(inputs["Wq"], dtype=np.float32)
    Wk = np.asarray(inputs["Wk"], dtype=np.float32)
    Wv = np.asarray(inputs["Wv"], dtype=np.float32)
    Wo = np.asarray(inputs["Wo"], dtype=np.float32)
    bq = np.asarray(inputs["bq"], dtype=np.float32)
    bv = np.asarray(inputs["bv"], dtype=np.float32)
    bo = np.asarray(inputs["bo"], dtype=np.float32)

    nc = _get_nc()

    in_maps = []
    for c in range(8):
        b, hg = c // 2, c % 2
        cs = slice(hg * HO, (hg + 1) * HO)
        xT = np.ascontiguousarray(x[b].T)               # [768, 2048]
        x8f = xT.astype(E4NP)
        xrf = (16.0 * (xT - x8f.astype(np.float32))).astype(E4NP)
        m = {"x8": _dr_rows_x(x8f), "xr": _dr_rows_x(xrf)}
        for t, W in (("q", Wq), ("k", Wk), ("v", Wv)):
            Ws = W[:, cs]
            for v, arr in zip(("a", "b", "r"), _w_tensors(Ws)):
                m[f"w{t}{v}"] = arr
        Wos = Wo[cs, :]                                  # [384, 768]
        m["wo"] = np.ascontiguousarray(
            Wos.reshape(MC, P, D).transpose(1, 0, 2).reshape(P, MC * D)
        ).astype(np.float16)
        m["bq"] = np.ascontiguousarray(256.0 * bq[cs])
        in_maps.append(m)

    res = run_bass_kernel_spmd(nc, in_maps, core_ids=list(range(8)))
    bias_full = bv @ Wo + bo
    out = np.empty((B, L, D), dtype=np.float32)
    for b in range(B):
        out[b] = (
            res.results[2 * b]["y"].astype(np.float32)
            + res.results[2 * b + 1]["y"].astype(np.float32)
        ) / 256.0 + bias_full
    return out


# revision 30
# speedup vs baseline: 1.0053x; 1.0040x over previous
"""Multi-head attention (B=4, L=2048, D=768, H=12) on 8 Trainium2 NeuronCores.

Sharding: (batch, head-group). Core c handles batch c//2 and heads
6*(c%2) .. 6*(c%2)+6.  Each core computes its 6 heads' attention output and
the partial output projection y_part = AO @ Wo[rows]; the host sums the two
partials per batch, rescales, and adds biases.  No collectives.

Per-core pipeline (fp16 operands; fp32 PSUM accumulate):
  A. QKV projections in fp8e4m3 DoubleRow with an exact 3-term residual
     decomposition (all terms at x256 scale, one PSUM group each):
        256*x@W = x8@W256 + xr16@W16 + x8@Wr
     where x8=fp8(x), xr16=fp8(16(x-x8)), W256=fp8(256W), W16=fp8(16W),
     Wr=fp8(256W - W256).  Eviction converts to fp16 (Q adds 256*bq; bk is
     dropped -- softmax shift-invariant; bv/bo folded on the host).
  B. Scores in fp16: S^T[k,q] = K_h @ Q_h^T -> PSUM [128, 512];
     p = exp(s/8 - 2.5) in fp16, computed on ACT (real exp) or DVE
     (Schraudolph: the uint16 bits ARE the fp16 pattern; uint16 saturation
     at 0 handles underflow), balanced by running engine-load accumulators.
     PV runs query-major: pv[q, qc8, 0:64] accumulated over the 16 key
     chunks of one query-half; denominators via ones-matmuls in their own
     PSUM bank.  AO = 256*attn in fp16.
  C. Two query-half sweeps (qc 0-7 then 8-15) over all heads, so the AO
     transpose (XBAR) + output projection + y DMA of the first half overlap
     the second sweep.  y is fp16 at 256x; host: (y0+y1)/256 + bv@Wo + bo.
  D. A long run of tiny dummy matmuls at kernel start keeps PE busy through
     the p-state ramp until the first input DMAs land, so all real matmuls
     run at full clock (PE idle gaps reset the ramp).
"""

import numpy as np
import ml_dtypes

import concourse.mybir as mybir
import concourse.tile as tile
from concourse import bacc
from concourse.bass_utils import run_bass_kernel_spmd

F32 = mybir.dt.float32
F16 = mybir.dt.float16
U16 = mybir.dt.uint16
FP8 = mybir.dt.float8e4
P = 128
B, L, D, H = 4, 2048, 768, 12
HD = 64                    # head dim
HL = H // 2                # heads per core = 6
HO = HL * HD               # local feature dim = 384
KC = D // P                # contraction chunks over D = 6
CP = KC // 2               # DoubleRow contraction chunk-pairs = 3
LC = L // P                # key chunks = 16
MC = HO // P               # feature chunks = 3
QC = LC                    # query chunks = 16
DR = mybir.MatmulPerfMode.DoubleRow

LOG2E = 1.4426950408889634
SHIFT = 2.5
# scores psum = (256Q)*(256K) = 65536 * s;  p = exp(s/8 - SHIFT)
ACT_SCALE = 0.125 / 65536.0
SCH_A = 1024.0 * LOG2E / (8.0 * 65536.0)
SCH_C = -60.0
SCH_B = 15 * 1024 - SHIFT * LOG2E * 1024.0 + SCH_C

PV_LAG = 8                 # PV trails scores by this many lk-units
N_WARMUP = 160             # tiny dummy matmuls bridging the p-state ramp

# engine time models (ns) for load balancing
ACT_EXP = 570.0
DVE_EXP = 658.0

_NC = None


def s512(i):
    return slice(i * 512, (i + 1) * 512)


def build():
    nc = bacc.Bacc("TRN2", target_bir_lowering=False, debug=False)

    # x blocks of 512 queries, contiguous per (partition, block)
    x8 = nc.dram_tensor("x8", [P, 4, CP, 2, 512], FP8, kind="ExternalInput")
    xr = nc.dram_tensor("xr", [P, 4, CP, 2, 512], FP8, kind="ExternalInput")
    w_names = []
    for t in ("q", "k", "v"):
        for v in ("a", "b", "r"):       # a=W256, b=W16, r=Wr
            w_names.append(f"w{t}{v}")
    w_dram = {
        n: nc.dram_tensor(n, [P, CP * 2 * HO], FP8, kind="ExternalInput")
        for n in w_names
    }
    wo = nc.dram_tensor("wo", [P, MC * D], F16, kind="ExternalInput")
    bq = nc.dram_tensor("bq", [HO], F32, kind="ExternalInput")
    y = nc.dram_tensor("y", [L, D], F16, kind="ExternalOutput")

    with tile.TileContext(nc) as tc:
        with tc.tile_pool(name="static", bufs=1) as static:
            qT_tiles = [static.tile([P, L], F16, name=f"qT{m}") for m in range(MC)]
            kT_tiles = [static.tile([P, L], F16, name=f"kT{m}") for m in range(MC)]
            v_sb = static.tile([P, LC, HL, HD], F16)
            ones_sb = static.tile([P, 1], F16)
            ao_q = static.tile([P, QC, HL, HD], F16)      # query-major AO
            ao_t = static.tile([P, QC, MC, P], F16)       # feature-major AO
            bq_sb = static.tile([P, MC], F32)
            shift_sb = static.tile([P, 1], F32)
            dummy_sb = static.tile([P, 64], F16)
            dummy_ps_out = static.tile([P, 1], F32)

            nc.vector.memset(dummy_sb[:], 0.0)

            with (
                tc.tile_pool(name="xpool", bufs=1) as xpool,
                tc.tile_pool(name="wpool", bufs=1) as wpool,
            ):
                x8_sb = xpool.tile([P, 4, CP, 2, 512], FP8, name="x8")
                xr_sb = xpool.tile([P, 4, CP, 2, 512], FP8, name="xr")
                w_sb = {
                    n: wpool.tile([P, CP, 2, HO], FP8, name=n) for n in w_names
                }
                wo_sb = wpool.tile([P, MC, D], F16, name="wo")

                def w_dma(n):
                    nc.sync.dma_start(
                        w_sb[n][:],
                        w_dram[n].ap().rearrange(
                            "p (c t h) -> p c t h", c=CP, t=2
                        ),
                    )

                # DMA order matches prelude consumption:
                #   Q00 K00 Q01 K01 V0 V1 Q02 K02 Q03 K03
                w_dma("wqa")
                nc.sync.dma_start(x8_sb[:, 0], x8[:, 0])
                nc.sync.dma_start(xr_sb[:, 0], xr[:, 0])
                w_dma("wqb")
                w_dma("wqr")
                for v in ("a", "b", "r"):
                    w_dma(f"wk{v}")
                nc.sync.dma_start(x8_sb[:, 1], x8[:, 1])
                nc.sync.dma_start(xr_sb[:, 1], xr[:, 1])
                for v in ("a", "b", "r"):
                    w_dma(f"wv{v}")
                for j in (2, 3):
                    nc.sync.dma_start(x8_sb[:, j], x8[:, j])
                    nc.sync.dma_start(xr_sb[:, j], xr[:, j])
                nc.sync.dma_start(bq_sb[:], bq.ap().rearrange("(c p) -> p c", p=P))
                nc.sync.dma_start(
                    wo_sb[:, :, :], wo.ap().rearrange("p (c d) -> p c d", c=MC)
                )

                with (
                    tc.tile_pool(name="ppool", bufs=12) as ppool,
                    tc.tile_pool(name="rpool", bufs=4) as rpool,
                    tc.tile_pool(name="ypool", bufs=4) as ypool,
                    tc.tile_pool(name="sps", bufs=6, space="PSUM") as sps,
                    tc.tile_pool(name="pvps", bufs=1, space="PSUM") as pvps,
                    tc.tile_pool(name="dnps", bufs=1, space="PSUM") as dnps,
                ):
                    # ---- PE p-state warm-up: tiny dummy matmuls keep PE
                    # busy until the first input DMAs land (idle resets
                    # the ramp, so bridge the whole window) ----
                    dn_full = dnps.tile([P, 2, 256], F32, name="dn")
                    nc.vector.memset(dn_full[0:64, 0, 128:192], 0.0)

                    def dummies(n):
                        # keep PE busy through DMA waits: accumulate zeros
                        # into a spare never-started region of the dn bank
                        for _ in range(n):
                            nc.tensor.matmul(
                                dn_full[0:64, 0, 128:192],
                                dummy_sb[:, :],
                                dummy_sb[:, :],
                                start=False,
                                stop=False,
                                skip_group_check=True,
                            )

                    dummies(N_WARMUP)
                    # deferred setup (not needed until the first exp/PV)
                    nc.vector.memset(dn_full[:, :, 0:8], 0.0)
                    nc.vector.memset(ones_sb[:], 0.0625)  # dn = sum(p)/16
                    nc.vector.memset(shift_sb[:], -SHIFT)
                    # preload the exp activation table while DMAs run
                    nc.scalar.activation(
                        out=dummy_ps_out[:],
                        in_=shift_sb[:],
                        func=mybir.ActivationFunctionType.Exp,
                    )
                    # deferred setup (not needed until the first exp/PV)
                    nc.vector.memset(dn_full[:, :, 0:8], 0.0)
                    nc.vector.memset(ones_sb[:], 0.0625)  # dn = sum(p)/16
                    nc.vector.memset(shift_sb[:], -SHIFT)
                    # preload the exp activation table while DMAs run
                    nc.scalar.activation(
                        out=dummy_ps_out[:],
                        in_=shift_sb[:],
                        func=mybir.ActivationFunctionType.Exp,
                    )

                    # engine-balance accumulators (ns)
                    eng_load = {"A": 0.0, "D": 0.0}

                    def pick_engine(act_cost, dve_cost):
                        if eng_load["A"] + act_cost <= eng_load["D"] + dve_cost:
                            eng_load["A"] += act_cost
                            return "A"
                        eng_load["D"] += dve_cost
                        return "D"

                    def emit_proj(w3, msl, j, moving_x, out_w, out_cb,
                                  nout=512):
                        """3-term fp8 DR projection into one PSUM group."""
                        ps = sps.tile([P, 512], F32, tag="s", name="pj")
                        wa, wb, wr = w3
                        first = True
                        for wt, xt in ((wa, x8_sb), (wb, xr_sb), (wr, x8_sb)):
                            for cp in range(CP):
                                if moving_x:
                                    lhs = wt[:, cp, :, msl]
                                    rhs = xt[:, j, cp, :, :]
                                else:
                                    lhs = xt[:, j, cp, :, msl]
                                    rhs = wt[:, cp, :, :]
                                nc.tensor.matmul(
                                    ps[:, 0:nout],
                                    lhs,
                                    rhs,
                                    start=first,
                                    stop=(wt is wr and cp == CP - 1),
                                    perf_mode=DR,
                                    skip_group_check=True,
                                )
                                first = False
                        out_cb(ps)

                    def do_qk(which, m, j):
                        w3 = tuple(w_sb[f"w{which}{v}"] for v in ("a", "b", "r"))

                        def evict(ps):
                            out_ap = (qT_tiles if which == "q" else kT_tiles)[m][
                                :, s512(j)
                            ]
                            e = pick_engine(570.0, 658.0)
                            if which == "q":
                                if e == "A":
                                    nc.scalar.activation(
                                        out=out_ap,
                                        in_=ps[:, :],
                                        func=mybir.ActivationFunctionType.Identity,
                                        bias=bq_sb[:, m : m + 1],
                                    )
                                else:
                                    nc.vector.tensor_scalar(
                                        out_ap,
                                        ps[:, :],
                                        1.0,
                                        bq_sb[:, m : m + 1],
                                        mybir.AluOpType.mult,
                                        mybir.AluOpType.add,
                                    )
                            else:
                                if e == "A":
                                    nc.scalar.activation(
                                        out=out_ap,
                                        in_=ps[:, :],
                                        func=mybir.ActivationFunctionType.Copy,
                                    )
                                else:
                                    nc.vector.tensor_copy(out_ap, ps[:, :])

                        emit_proj(w3, slice(m * P, (m + 1) * P), j, True, None,
                                  evict)

                    def do_v(l):
                        w3 = tuple(w_sb[f"wv{v}"] for v in ("a", "b", "r"))

                        def evict(ps):
                            e = pick_engine(463.0, 525.0)
                            dst = v_sb[:, l, :, :]
                            src = ps[:, 0:HO].rearrange("p (h d) -> p h d", d=HD)
                            if e == "A":
                                nc.scalar.activation(
                                    out=dst, in_=src,
                                    func=mybir.ActivationFunctionType.Copy,
                                )
                            else:
                                nc.vector.tensor_copy(dst, src)

                        emit_proj(w3, slice((l % 4) * P, (l % 4 + 1) * P),
                                  l // 4, False, None, evict, nout=HO)

                    def emit_scores_exp(hl, lk, q4):
                        pc, odd = hl // 2, hl % 2
                        r0 = odd * HD
                        s_t = sps.tile([P, 512], F32, tag="s", name="sc")
                        nc.tensor.matmul(
                            s_t[:, :],
                            kT_tiles[pc][r0 : r0 + HD, lk * P : (lk + 1) * P],
                            qT_tiles[pc][r0 : r0 + HD, s512(q4)],
                            start=True,
                            stop=True,
                        )
                        p_t = ppool.tile([P, 512], F16, tag="p", name="pt")
                        if pick_engine(ACT_EXP, DVE_EXP) == "A":
                            nc.scalar.activation(
                                out=p_t[:, :],
                                in_=s_t[:, :],
                                func=mybir.ActivationFunctionType.Exp,
                                bias=shift_sb[:, 0:1],
                                scale=ACT_SCALE,
                            )
                        else:
                            nc.vector.tensor_scalar(
                                p_t[:, :].bitcast(U16),
                                s_t[:, :],
                                SCH_A,
                                SCH_B,
                                mybir.AluOpType.mult,
                                mybir.AluOpType.add,
                            )
                        return p_t

                    def emit_pv(pv, hl, lk, q4, p_t, sweep):
                        first = lk == 0 and q4 % 2 == 0
                        last = lk == LC - 1
                        for jj in range(4):
                            qc8 = (q4 % 2) * 4 + jj
                            nc.tensor.matmul(
                                pv[:, qc8, :],
                                p_t[:, jj * P : (jj + 1) * P],
                                v_sb[:, lk, hl, :],
                                start=(first and jj == 0),
                                stop=last,
                                skip_group_check=True,
                            )
                            nc.tensor.matmul(
                                dn_full[:, hl % 2, qc8 : qc8 + 1],
                                p_t[:, jj * P : (jj + 1) * P],
                                ones_sb[:, :],
                                start=False,
                                stop=last,
                                skip_group_check=True,
                            )

                    def evict_ao(pv, hl, sweep, split=False):
                        """recip + batched scaled eviction (256*attn, fp16)."""
                        rstage = rpool.tile([P, 8], F32, tag="r")
                        nc.vector.reciprocal(rstage[:, :], dn_full[:, hl % 2, 0:8])
                        eng_load["D"] += 135.0
                        if hl + 2 < HL or sweep == 0:
                            nc.vector.memset(dn_full[:, hl % 2, 0:8], 0.0)
                        qc0 = sweep * 8
                        groups = ((0, 4), (4, 8)) if split else ((0, 8),)
                        for g0, g1 in groups:
                            rb = rstage[:, g0:g1, None].broadcast_to(
                                (P, g1 - g0, HD))
                            nc.vector.scalar_tensor_tensor(
                                ao_q[:, qc0 + g0 : qc0 + g1, hl, :],
                                pv[:, g0:g1, :],
                                0.0625,
                                rb,
                                mybir.AluOpType.mult,
                                mybir.AluOpType.mult,
                            )
                            eng_load["D"] += 658.0 / len(groups)
                            if split:
                                quad_transpose(2 + (g0 // 4))

                    def quad_transpose(g):
                        nc.sync.dma_start_transpose(
                            ao_t[:, 4 * g : 4 * g + 4, :, :],
                            ao_q[:, 4 * g : 4 * g + 4, :, :],
                        )

                    def do_outproj(m):
                        y_t = ypool.tile([P, D], F16, tag="yt")
                        tail2 = m >= QC - 2
                        for half, (n0, nsz) in enumerate(((0, 512), (512, 256))):
                            ps = sps.tile([P, 512], F32, tag="s", name="yp")
                            for c in range(MC):
                                nc.tensor.matmul(
                                    ps[:, 0:nsz],
                                    ao_t[:, m, c, :],
                                    wo_sb[:, c, n0 : n0 + nsz],
                                    start=(c == 0),
                                    stop=(c == MC - 1),
                                )
                            e = pick_engine(
                                (nsz + 172) * 0.8333, (nsz + 120) * 1.0417)
                            if e == "A":
                                nc.scalar.activation(
                                    out=y_t[:, n0 : n0 + nsz], in_=ps[:, 0:nsz],
                                    func=mybir.ActivationFunctionType.Copy,
                                )
                            else:
                                nc.vector.tensor_copy(
                                    y_t[:, n0 : n0 + nsz], ps[:, 0:nsz])
                            if tail2:
                                yeng = nc.sync if half == 0 else nc.scalar
                                yeng.dma_start(
                                    y[m * P : (m + 1) * P, n0 : n0 + nsz],
                                    y_t[:, n0 : n0 + nsz],
                                )
                        if not tail2:
                            yeng = nc.sync if m % 2 == 0 else nc.scalar
                            yeng.dma_start(y[m * P : (m + 1) * P, :], y_t[:])

                    # ---------------- prelude ----------------
                    do_qk("q", 0, 0)
                    do_qk("k", 0, 0)
                    for job in (
                        lambda: do_qk("q", 0, 1), lambda: do_qk("k", 0, 1),
                        lambda: do_v(0), lambda: do_v(1),
                        lambda: do_qk("q", 0, 2), lambda: do_qk("k", 0, 2),
                        lambda: do_qk("q", 0, 3), lambda: do_qk("k", 0, 3),
                    ):
                        dummies(10)
                        job()

                    prejobs = {}
                    for j in range(4):                     # qT/kT chunk 1
                        prejobs.setdefault(4 * j, []).append(
                            lambda j=j: do_qk("q", 1, j))
                        prejobs.setdefault(4 * j + 2, []).append(
                            lambda j=j: do_qk("k", 1, j))
                    for l in range(2, LC):                 # V chunks 2-15
                        prejobs.setdefault(l - 1, []).append(lambda l=l: do_v(l))
                    for j in range(4):                     # qT/kT chunk 2
                        prejobs.setdefault(32 + 4 * j, []).append(
                            lambda j=j: do_qk("q", 2, j))
                        prejobs.setdefault(32 + 4 * j + 2, []).append(
                            lambda j=j: do_qk("k", 2, j))
                    # sweep B: transposes of qc 0-7 (after the (s1,h0) pump
                    # evicts (s0,h5)), then outproj jobs
                    t0 = 192 + PV_LAG + 3
                    prejobs.setdefault(t0, []).append(lambda: quad_transpose(0))
                    prejobs.setdefault(t0 + 2, []).append(
                        lambda: quad_transpose(1))
                    for i in range(8):                     # outproj qc 0-7
                        prejobs.setdefault(t0 + 16 + 16 * i, []).append(
                            lambda i=i: do_outproj(i))

                    units = []
                    for sweep in range(2):
                        for hl in range(HL):
                            if sweep == 1 and hl == HL - 1:
                                # final head: q4h-major so qc 8-11 finish
                                # early and their evict/transpose/outproj
                                # overlap the qc 12-15 stream
                                for q4h in range(2):
                                    for lk in range(LC):
                                        units.append((sweep, hl, lk, q4h))
                            else:
                                for lk in range(LC):
                                    for q4h in range(2):
                                        units.append((sweep, hl, lk, q4h))
                    pend = []
                    pv_state = {"tile": None, "key": None}

                    def evict_ao_part(pv, hl, sweep, g0, g1):
                        rstage = rpool.tile([P, 4], F32, tag="r")
                        nc.vector.reciprocal(
                            rstage[:, :], dn_full[:, hl % 2, g0:g1])
                        qc0 = sweep * 8
                        rb = rstage[:, :, None].broadcast_to((P, g1 - g0, HD))
                        nc.vector.scalar_tensor_tensor(
                            ao_q[:, qc0 + g0 : qc0 + g1, hl, :],
                            pv[:, g0:g1, :],
                            0.0625,
                            rb,
                            mybir.AluOpType.mult,
                            mybir.AluOpType.mult,
                        )

                    def pump_pv(entry):
                        hl, lk, q4, p_t, sweep = entry
                        key = (sweep, hl)
                        if pv_state["key"] != key:
                            if pv_state["tile"] is not None:
                                osweep, ohl = pv_state["key"]
                                evict_ao(pv_state["tile"], ohl, osweep)
                            pv_state["tile"] = pvps.tile(
                                [P, 8, HD], F32, tag="pv", name="pv")
                            pv_state["key"] = key
                        emit_pv(pv_state["tile"], hl, lk, q4, p_t, sweep)
                        if (sweep, hl, lk, q4) == (1, HL - 1, LC - 1, 2):
                            # qc 8-11 of the final head are complete: evict
                            # and transpose now, overlapping the qc 12-15
                            # stream
                            evict_ao_part(pv_state["tile"], HL - 1, 1, 0, 4)
                            quad_transpose(2)

                    for u, (sweep, hl, lk, q4h) in enumerate(units):
                        q4 = 2 * sweep + q4h
                        if u < 16 and u % 2 == 1:
                            dummies(8)
                        for job in prejobs.get(u, ()):
                            job()
                        p_t = emit_scores_exp(hl, lk, q4)
                        pend.append((hl, lk, q4, p_t, sweep))
                        while len(pend) > PV_LAG:
                            pump_pv(pend.pop(0))
                    while pend:
                        pump_pv(pend.pop(0))
                    # tail: evict qc 12-15, then outproj 8-11 (transpose(2)
                    # already done) runs during transpose(3)'s DMA latency
                    evict_ao_part(pv_state["tile"], HL - 1, 1, 4, 8)
                    quad_transpose(3)
                    for i in range(8, QC):
                        do_outproj(i)

    nc.compile()
    return nc


def _get_nc():
    global _NC
    if _NC is None:
        _NC = build()
    return _NC


E4NP = ml_dtypes.float8_e4m3


def _dr_rows_x(a):
    """[768, 2048] -> [128, 4, 3, 2, 512]: query-block-major DR layout;
    row (cp, t, p) holds input row cp*256 + t*128 + p."""
    return np.ascontiguousarray(
        a.reshape(CP, 2, P, 4, 512).transpose(2, 3, 0, 1, 4)
    )


def _w_tensors(W):
    """W [768, 384] fp32 -> (W256, W16, Wr) fp8, flattened [128, 3*2*384]
    with row (cp, t, p) holding input row cp*256 + t*128 + p."""
    Wa = (256.0 * W).astype(E4NP)
    Wb = (16.0 * W).astype(E4NP)
    Wr = (256.0 * W - Wa.astype(np.float32)).astype(E4NP)
    return tuple(
        np.ascontiguousarray(
            t.reshape(CP, 2, P, HO).transpose(2, 0, 1, 3).reshape(P, CP * 2 * HO)
        )
        for t in (Wa, Wb, Wr)
    )


def kernel(**inputs) -> np.ndarray:
    x = np.asarray(inputs["x"], dtype=np.float32)
    Wq = np.asarray(inputs["Wq"], dtype=np.float32)
    Wk = np.asarray(inputs["Wk"], dtype=np.float32)
    Wv = np.asarray(inputs["Wv"], dtype=np.float32)
    Wo = np.asarray(inputs["Wo"], dtype=np.float32)
    bq = np.asarray(inputs["bq"], dtype=np.float32)
    bv = np.asarray(inputs["bv"], dtype=np.float32)
    bo = np.asarray(inputs["bo"], dtype=np.float32)

    nc = _get_nc()

    in_maps = []
    for c in range(8):
        b, hg = c // 2, c % 2
        cs = slice(hg * HO, (hg + 1) * HO)
        xT = np.ascontiguousarray(x[b].T)               # [768, 2048]
        x8f = xT.astype(E4NP)
        xrf = (16.0 * (xT - x8f.astype(np.float32))).astype(E4NP)
        m = {"x8": _dr_rows_x(x8f), "xr": _dr_rows_x(xrf)}
        for t, W in (("q", Wq), ("k", Wk), ("v", Wv)):
            Ws = W[:, cs]
            for v, arr in zip(("a", "b", "r"), _w_tensors(Ws)):
                m[f"w{t}{v}"] = arr
        Wos = Wo[cs, :]                                  # [384, 768]
        m["wo"] = np.ascontiguousarray(
            Wos.reshape(MC, P, D).transpose(1, 0, 2).reshape(P, MC * D)
        ).astype(np.float16)
        m["bq"] = np.ascontiguousarray(256.0 * bq[cs])
        in_maps.append(m)

    res = run_bass_kernel_spmd(nc, in_maps, core_ids=list(range(8)))
    bias_full = bv @ Wo + bo
    out = np.empty((B, L, D), dtype=np.float32)
    for b in range(B):
        out[b] = (
            res.results[2 * b]["y"].astype(np.float32)
            + res.results[2 * b + 1]["y"].astype(np.float32)
        ) / 256.0 + bias_full
    return out
